# revision 19
# baseline (speedup 1.0000x reference)
"""Trainium2 Bass kernel: unnormalized single-head attention block.

Computes, for x [4, 4096, 1024] and w_q/w_k/w_v/w_o [1024, 1024] (all fp32):
    q = x @ w_q ; k = x @ w_k ; v = x @ w_v
    scores = q @ k.T            (no softmax)
    out = (scores @ v) @ w_o

There is no softmax, so matmul associativity applies:
    out_b = x_b @ (w_q @ w_k.T @ G_b @ w_v @ w_o),   G_b = x_b.T @ x_b
which drops the arithmetic from ~413 GFLOP (two [T,T] products) to ~90 GFLOP.
G is symmetric, so only its upper-triangle 128-blocks are computed directly;
the lower blocks are PE transposes of the upper ones (~3us vs ~48us of MMs).
The weight chain is right-associated against the core's 512-column slice of
w_o, so every factor is a [D,D] x [D,512] product (4 x 13.7us; no [D,D]x[D,D]
products at all):
    M2 = wq @ (wk.T @ (G @ (wv @ woh)))

Sharding: 8 NeuronCores = (4 batches) x (2 output-column halves). Each core
computes G_b over the full sequence, its M2 slice, and out[:, half] = x @ M2.
No collectives, no inter-core deps. (A pair-AllReduce version that halves the
G work was measured SLOWER: the CC op costs ~29us at ~40GB/s bus plus ~25us
of startup latency, which cannot hide behind the ~41us of independent work.)

Device math is bf16 (host-cast) with fp32 PSUM accumulation. Layout chain
(lhsT's partition dim is always the contraction dim):
    G[d,e]   = sum_t  xn[t,d]  xn[t,e]      lhsT=xn tile,   rhs=xn tile
    N1[e,f]  = sum_k  wvT[k,e] woh[k,f]     lhsT=wvT,       rhs=woh
    N2[d,f]  = sum_e  G[d,e]   N1[e,f]      lhsT=G (sym),   rhs=N1
    K1[c,f]  = sum_d  wk[d,c]  N2[d,f]      lhsT=wk,        rhs=N2
    M2[i,f]  = sum_c  wqT[c,i] K1[c,f]      lhsT=wqT,       rhs=K1
    out[t,f] = sum_i  xt[i,t]  M2[i,f]      lhsT=xt tile,   rhs=M2
"""

import contextlib
import ctypes
import os
import sys
import types

import numpy as np

B = 4
T = 4096
D = 1024
P = 128             # SBUF partitions
NCORES = 8
DT = D // P         # 8 tiles along any 1024 dim
ST = T // P         # 32 tiles along the sequence
FREE = 512          # PSUM bank width (fp32)
FH = D // 2         # 512 output columns per core

# Upper-triangle chunk table for symmetric G: (jt, psum chunk, e-start, width).
G_CHUNKS = []
for _jt in range(DT):
    for _c in range(2):
        _es = max(_c * FREE, _jt * P)
        _w = (_c + 1) * FREE - _es
        if _w > 0:
            G_CHUNKS.append((_jt, _c, _es, _w))
G_PASS = [[ch for ch in G_CHUNKS if ch[0] < 4], [ch for ch in G_CHUNKS if ch[0] >= 4]]

_STATE = {}
LAST_RESULTS = None


def _install_axon_ntff_shim():
    """bass_utils(trace=True) under axon imports antenv.axon_hooks, which the
    agent image lacks. Provide the documented ctypes equivalent so tracing
    works; degrades to hook=None when the .so has no profile symbols."""
    try:
        import antenv.axon_hooks  # noqa: F401
        return
    except ImportError:
        pass

    so_path = "/opt/axon/libaxon_pjrt.so"

    def _make_hook():
        try:
            lib = ctypes.CDLL(so_path)
        except OSError:
            return None
        if not hasattr(lib, "axon_start_nrt_profile"):
            return None
        lib.axon_start_nrt_profile.argtypes = [
            ctypes.POINTER(ctypes.c_int64),
            ctypes.c_size_t,
        ]
        lib.axon_start_nrt_profile.restype = ctypes.c_int64
        lib.axon_stop_nrt_profile.argtypes = [ctypes.c_char_p]
        lib.axon_stop_nrt_profile.restype = ctypes.c_int64

        @contextlib.contextmanager
        def _hook(output_dir, device_ids):
            import jax

            jax.devices()
            if device_ids:
                ids = (ctypes.c_int64 * len(device_ids))(*device_ids)
                rc = lib.axon_start_nrt_profile(ids, len(device_ids))
            else:
                rc = lib.axon_start_nrt_profile(None, 0)
            if rc != 0:
                raise RuntimeError(f"axon_start_nrt_profile rc={rc}")
            try:
                yield
            finally:
                n = lib.axon_stop_nrt_profile(str(output_dir).encode())
                print(f"profile: {n} file(s) written to {output_dir}", file=sys.stderr)

        return _hook

    mod = types.ModuleType("antenv.axon_hooks")
    mod.get_axon_ntff_profile_hook = _make_hook
    mod.set_axon_ntff_profile_hook = lambda h: None
    sys.modules["antenv.axon_hooks"] = mod


def _trace_kernel(tc, xn, xt, wqT, wk, wvT, woh, ident, out):
    import concourse.mybir as mybir
    from concourse.bass import ts

    nc = tc.nc
    f32 = mybir.dt.float32
    bf16 = mybir.dt.bfloat16

    with contextlib.ExitStack() as top:
        gsb_pool = top.enter_context(tc.tile_pool(name="gsb", bufs=DT))
        n1_pool = top.enter_context(tc.tile_pool(name="n1", bufs=DT))
        n2_pool = top.enter_context(tc.tile_pool(name="n2", bufs=DT))
        k1_pool = top.enter_context(tc.tile_pool(name="k1", bufs=DT))
        m2_pool = top.enter_context(tc.tile_pool(name="m2", bufs=DT))
        id_pool = top.enter_context(tc.tile_pool(name="idp", bufs=1))
        wq_pool = top.enter_context(tc.tile_pool(name="wq", bufs=DT))
        wk_pool = top.enter_context(tc.tile_pool(name="wk", bufs=DT))
        wv_pool = top.enter_context(tc.tile_pool(name="wv", bufs=DT))
        wo_pool = top.enter_context(tc.tile_pool(name="wo", bufs=DT))
        xt_pool = top.enter_context(tc.tile_pool(name="xt", bufs=DT))
        ost_pool = top.enter_context(tc.tile_pool(name="ost", bufs=8))
        ps_pool = top.enter_context(tc.tile_pool(name="ps", bufs=8, space="PSUM"))

        gsb = [gsb_pool.tile([P, D], bf16, name=f"g{i}", tag="gsb") for i in range(DT)]
        n1 = [n1_pool.tile([P, FH], bf16, name=f"n1_{i}", tag="n1") for i in range(DT)]
        n2 = [n2_pool.tile([P, FH], bf16, name=f"n2_{i}", tag="n2") for i in range(DT)]
        k1 = [k1_pool.tile([P, FH], bf16, name=f"k1_{i}", tag="k1") for i in range(DT)]
        m2 = [m2_pool.tile([P, FH], bf16, name=f"m2_{i}", tag="m2") for i in range(DT)]
        idt = id_pool.tile([P, P], bf16, name="idt", tag="idt")

        # DMA queues: gpsimd carries wvT then xt group 0 (N1 is the tensor
        # engine's opening act); woh rides sync ahead of xn; xn splits across
        # sync/scalar so G is never DMA-starved; wk/wqT follow behind. xt
        # groups 1-3 recycle xn's SBUF space once G is done.
        wvb = [wv_pool.tile([P, D], bf16, name=f"wv{i}", tag="wv") for i in range(DT)]
        wob = [wo_pool.tile([P, FH], bf16, name=f"wo{i}", tag="wo") for i in range(DT)]
        for i in range(DT):
            nc.gpsimd.dma_start(out=wvb[i][:], in_=wvT[ts(i, P), :])
            nc.sync.dma_start(out=wob[i][:], in_=woh[ts(i, P), :])
        nc.gpsimd.dma_start(out=idt[:], in_=ident)  # only needed at ~100us
        xtg_all = [
            [xt_pool.tile([P, D], bf16, name=f"xt0_{i}", tag="xt") for i in range(DT)]
        ]
        for it in range(DT):
            nc.gpsimd.dma_start(out=xtg_all[0][it][:], in_=xt[ts(it, P), ts(0, D)])

        with contextlib.ExitStack() as setup:
            xn_pool = setup.enter_context(tc.tile_pool(name="xn", bufs=ST))
            xnb = [xn_pool.tile([P, D], bf16, name=f"xn{i}", tag="xn") for i in range(ST)]
            for i in range(ST):
                q = nc.sync if i % 2 == 0 else nc.scalar
                q.dma_start(out=xnb[i][:], in_=xn[ts(i, P), :])
            wkb = [wk_pool.tile([P, D], bf16, name=f"wk{i}", tag="wk") for i in range(DT)]
            wqb = [wq_pool.tile([P, D], bf16, name=f"wq{i}", tag="wq") for i in range(DT)]
            for i in range(DT):
                nc.sync.dma_start(out=wkb[i][:], in_=wk[ts(i, P), :])
                nc.scalar.dma_start(out=wqb[i][:], in_=wqT[ts(i, P), :])

            # --- N1 = wv @ wo[:, half] (tensor warms up while xn streams) ---
            pss = [ps_pool.tile([P, FREE], f32, name="psn1", tag="ps") for _ in range(DT)]
            for dt in range(DT):
                for eb in range(DT):
                    nc.tensor.matmul(
                        pss[eb][:],
                        wvb[dt][:, ts(eb, P)],
                        wob[dt][:],
                        start=(dt == 0),
                        stop=(dt == DT - 1),
                    )
            # Alternate evac engines: G-A's first matmuls wait on these banks,
            # so draining them on one engine would serialize ~5.5us.
            for eb in range(DT):
                if eb % 2 == 0:
                    nc.vector.tensor_copy(n1[eb][:], pss[eb][:])
                else:
                    nc.scalar.copy(n1[eb][:], pss[eb][:])

            # --- G upper triangle: two streaming passes over the sequence ---
            for chunks in G_PASS:
                pss = {
                    (jt, c): ps_pool.tile([P, FREE], f32, name="psg", tag="ps")
                    for (jt, c, es, w) in chunks
                }
                for tt in range(ST):
                    for jt, c, es, w in chunks:
                        nc.tensor.matmul(
                            pss[jt, c][:, :w],
                            xnb[tt][:, ts(jt, P)],
                            xnb[tt][:, es : es + w],
                            start=(tt == 0),
                            stop=(tt == ST - 1),
                        )
                for jt, c, es, w in chunks:
                    nc.vector.tensor_copy(gsb[jt][:, es : es + w], pss[jt, c][:, :w])

            # --- mirror the lower-triangle blocks: G[jt,eb] = G[eb,jt].T ---
            for jt in range(1, DT):
                for eb in range(jt):
                    pst = ps_pool.tile([P, P], bf16, name="pst", tag="ps")
                    nc.tensor.transpose(pst[:], gsb[eb][:, ts(jt, P)], idt[:])
                    nc.vector.tensor_copy(gsb[jt][:, ts(eb, P)], pst[:])

        # xt groups 1-3 into the SBUF space xn vacated.
        xt2_pool = top.enter_context(tc.tile_pool(name="xt2", bufs=3 * DT))
        for g in range(1, 4):
            xtg = [
                xt2_pool.tile([P, D], bf16, name=f"xt{g}_{i}", tag="xt2")
                for i in range(DT)
            ]
            for it in range(DT):
                nc.gpsimd.dma_start(out=xtg[it][:], in_=xt[ts(it, P), ts(g, D)])
            xtg_all.append(xtg)

        # --- N2 = G @ N1 (lhsT=G works because G is symmetric) ---
        pss = [ps_pool.tile([P, FREE], f32, name="psn2", tag="ps") for _ in range(DT)]
        for et in range(DT):
            for db in range(DT):
                nc.tensor.matmul(
                    pss[db][:],
                    gsb[et][:, ts(db, P)],
                    n1[et][:],
                    start=(et == 0),
                    stop=(et == DT - 1),
                )
        for db in range(DT):
            nc.scalar.copy(n2[db][:], pss[db][:])

        # --- K1 = wk.T @ N2 ---
        pss = [ps_pool.tile([P, FREE], f32, name="psk1", tag="ps") for _ in range(DT)]
        for dt in range(DT):
            for cb in range(DT):
                nc.tensor.matmul(
                    pss[cb][:],
                    wkb[dt][:, ts(cb, P)],
                    n2[dt][:],
                    start=(dt == 0),
                    stop=(dt == DT - 1),
                )
        for cb in range(DT):
            nc.vector.tensor_copy(k1[cb][:], pss[cb][:])

        # --- M2 = wq @ K1 ---
        pss = [ps_pool.tile([P, FREE], f32, name="psm2", tag="ps") for _ in range(DT)]
        for ct in range(DT):
            for ib in range(DT):
                nc.tensor.matmul(
                    pss[ib][:],
                    wqb[ct][:, ts(ib, P)],
                    k1[ct][:],
                    start=(ct == 0),
                    stop=(ct == DT - 1),
                )
        for ib in range(DT):
            nc.scalar.copy(m2[ib][:], pss[ib][:])

        # --- out[:, half] = x @ M2, streaming xt column-groups of 1024 ---
        for g in range(4):
            xtg = xtg_all[g]
            pss = [ps_pool.tile([P, FREE], f32, name="pso", tag="ps") for _ in range(8)]
            for it in range(DT):
                for tb in range(8):
                    nc.tensor.matmul(
                        pss[tb][:],
                        xtg[it][:, ts(tb, P)],
                        m2[it][:],
                        start=(it == 0),
                        stop=(it == DT - 1),
                    )
            # Tail latency: alternate evac engines and store queues so the
            # last group's flush isn't serialized on one engine.
            for tb in range(8):
                ot = ost_pool.tile([P, FREE], f32, name="ot", tag="ost")
                if tb % 2 == 0:
                    nc.scalar.copy(ot[:], pss[tb][:])
                    nc.scalar.dma_start(out=out[ts(g * 8 + tb, P), :], in_=ot[:])
                else:
                    nc.vector.tensor_copy(ot[:], pss[tb][:])
                    nc.sync.dma_start(out=out[ts(g * 8 + tb, P), :], in_=ot[:])


def _build():
    _install_axon_ntff_shim()
    import concourse.mybir as mybir
    import concourse.tile as tile
    from concourse import bacc

    f32 = mybir.dt.float32
    bf16 = mybir.dt.bfloat16
    nc = bacc.Bacc("TRN2", target_bir_lowering=False, debug=False, num_devices=NCORES)
    xn = nc.dram_tensor("xn", [T, D], bf16, kind="ExternalInput").ap()
    xt = nc.dram_tensor("xt", [D, T], bf16, kind="ExternalInput").ap()
    wqT = nc.dram_tensor("wqT", [D, D], bf16, kind="ExternalInput").ap()
    wk = nc.dram_tensor("wk", [D, D], bf16, kind="ExternalInput").ap()
    wvT = nc.dram_tensor("wvT", [D, D], bf16, kind="ExternalInput").ap()
    woh = nc.dram_tensor("woh", [D, FH], bf16, kind="ExternalInput").ap()
    ident = nc.dram_tensor("ident", [P, P], bf16, kind="ExternalInput").ap()
    out = nc.dram_tensor("out", [T, FH], f32, kind="ExternalOutput").ap()

    with tile.TileContext(nc) as tc:
        _trace_kernel(tc, xn, xt, wqT, wk, wvT, woh, ident, out)
    nc.compile()
    return nc


def kernel(x, w_q, w_k, w_v, w_o):
    global LAST_RESULTS
    import ml_dtypes
    from concourse import bass_utils

    bf16 = ml_dtypes.bfloat16

    if "nc" not in _STATE:
        _STATE["nc"] = _build()
    nc = _STATE["nc"]

    x = np.ascontiguousarray(x, dtype=np.float32)
    wqT = np.ascontiguousarray(np.asarray(w_q, dtype=np.float32).T).astype(bf16)
    wkb = np.ascontiguousarray(np.asarray(w_k, dtype=np.float32)).astype(bf16)
    wvT = np.ascontiguousarray(np.asarray(w_v, dtype=np.float32).T).astype(bf16)
    wo = np.asarray(w_o, dtype=np.float32)
    wo_halves = [
        np.ascontiguousarray(wo[:, :FH]).astype(bf16),
        np.ascontiguousarray(wo[:, FH:]).astype(bf16),
    ]
    ident = np.eye(P, dtype=np.float32).astype(bf16)
    xn_b = [x[b].astype(bf16) for b in range(B)]
    xt_b = [np.ascontiguousarray(x[b].T).astype(bf16) for b in range(B)]

    in_maps = []
    for core in range(NCORES):
        b, fh = core // 2, core % 2
        in_maps.append(
            {
                "xn": xn_b[b],
                "xt": xt_b[b],
                "wqT": wqT,
                "wk": wkb,
                "wvT": wvT,
                "woh": wo_halves[fh],
                "ident": ident,
            }
        )

    LAST_RESULTS = bass_utils.run_bass_kernel_spmd(
        nc, in_maps, core_ids=list(range(NCORES))
    )
    out = np.empty((B, T, D), dtype=np.float32)
    for core in range(NCORES):
        b, fh = core // 2, core % 2
        out[b, :, fh * FH : (fh + 1) * FH] = LAST_RESULTS.results[core]["out"]
    return out


# revision 22
# speedup vs baseline: 1.1988x; 1.1988x over previous
"""Trainium2 Bass kernel: unnormalized single-head attention block.

Computes, for x [4, 4096, 1024] and w_q/w_k/w_v/w_o [1024, 1024] (all fp32):
    q = x @ w_q ; k = x @ w_k ; v = x @ w_v
    scores = q @ k.T            (no softmax)
    out = (scores @ v) @ w_o

There is no softmax, so matmul associativity applies:
    out_b = x_b @ (w_q @ w_k.T @ G_b @ w_v @ w_o),   G_b = x_b.T @ x_b
which drops the arithmetic from ~413 GFLOP (two [T,T] products) to ~90 GFLOP.
G is symmetric, so only its upper-triangle 128-blocks are computed directly;
the lower blocks are PE transposes of the upper ones (~3us vs ~48us of MMs).
The weight chain is right-associated against the core's 512-column slice of
w_o, so every factor is a [D,D] x [D,512] product (4 x 13.7us; no [D,D]x[D,D]
products at all):
    M2 = wq @ (wk.T @ (G @ (wv @ woh)))

Sharding: 8 NeuronCores = (4 batches) x (2 output-column halves). Each core
computes G_b over the full sequence, its M2 slice, and out[:, half] = x @ M2.
No collectives, no inter-core deps. (A pair-AllReduce version that halves the
G work was measured SLOWER: the CC op costs ~29us at ~40GB/s bus plus ~25us
of startup latency, which cannot hide behind the ~41us of independent work.)

Device math is bf16 (host-cast) with fp32 PSUM accumulation. Layout chain
(lhsT's partition dim is always the contraction dim):
    G[d,e]   = sum_t  xn[t,d]  xn[t,e]      lhsT=xn tile,   rhs=xn tile
    N1[e,f]  = sum_k  wvT[k,e] woh[k,f]     lhsT=wvT,       rhs=woh
    N2[d,f]  = sum_e  G[d,e]   N1[e,f]      lhsT=G (sym),   rhs=N1
    K1[c,f]  = sum_d  wk[d,c]  N2[d,f]      lhsT=wk,        rhs=N2
    M2[i,f]  = sum_c  wqT[c,i] K1[c,f]      lhsT=wqT,       rhs=K1
    out[t,f] = sum_i  xt[i,t]  M2[i,f]      lhsT=xt tile,   rhs=M2
"""

import contextlib
import ctypes
import os
import sys
import types

import numpy as np

B = 4
T = 4096
D = 1024
P = 128             # SBUF partitions
NCORES = 8
DT = D // P         # 8 tiles along any 1024 dim
ST = T // P         # 32 tiles along the sequence
FREE = 512          # PSUM bank width (fp32)
FH = D // 2         # 512 output columns per core

# Upper-triangle chunk table for symmetric G: (jt, psum chunk, e-start, width).
G_CHUNKS = []
for _jt in range(DT):
    for _c in range(2):
        _es = max(_c * FREE, _jt * P)
        _w = (_c + 1) * FREE - _es
        if _w > 0:
            G_CHUNKS.append((_jt, _c, _es, _w))
G_PASS = [[ch for ch in G_CHUNKS if ch[0] < 4], [ch for ch in G_CHUNKS if ch[0] >= 4]]

_STATE = {}
LAST_RESULTS = None


def _install_axon_ntff_shim():
    """bass_utils(trace=True) under axon imports antenv.axon_hooks, which the
    agent image lacks. Provide the documented ctypes equivalent so tracing
    works; degrades to hook=None when the .so has no profile symbols."""
    try:
        import antenv.axon_hooks  # noqa: F401
        return
    except ImportError:
        pass

    so_path = "/opt/axon/libaxon_pjrt.so"

    def _make_hook():
        try:
            lib = ctypes.CDLL(so_path)
        except OSError:
            return None
        if not hasattr(lib, "axon_start_nrt_profile"):
            return None
        lib.axon_start_nrt_profile.argtypes = [
            ctypes.POINTER(ctypes.c_int64),
            ctypes.c_size_t,
        ]
        lib.axon_start_nrt_profile.restype = ctypes.c_int64
        lib.axon_stop_nrt_profile.argtypes = [ctypes.c_char_p]
        lib.axon_stop_nrt_profile.restype = ctypes.c_int64

        @contextlib.contextmanager
        def _hook(output_dir, device_ids):
            import jax

            jax.devices()
            if device_ids:
                ids = (ctypes.c_int64 * len(device_ids))(*device_ids)
                rc = lib.axon_start_nrt_profile(ids, len(device_ids))
            else:
                rc = lib.axon_start_nrt_profile(None, 0)
            if rc != 0:
                raise RuntimeError(f"axon_start_nrt_profile rc={rc}")
            try:
                yield
            finally:
                n = lib.axon_stop_nrt_profile(str(output_dir).encode())
                print(f"profile: {n} file(s) written to {output_dir}", file=sys.stderr)

        return _hook

    mod = types.ModuleType("antenv.axon_hooks")
    mod.get_axon_ntff_profile_hook = _make_hook
    mod.set_axon_ntff_profile_hook = lambda h: None
    sys.modules["antenv.axon_hooks"] = mod


def _trace_kernel(tc, xn, xt, wqT, wk, wvT, woh, ident, out):
    import concourse.mybir as mybir
    from concourse.bass import ts

    nc = tc.nc
    f32 = mybir.dt.float32
    bf16 = mybir.dt.bfloat16

    with contextlib.ExitStack() as top:
        gsb_pool = top.enter_context(tc.tile_pool(name="gsb", bufs=DT))
        n1_pool = top.enter_context(tc.tile_pool(name="n1", bufs=DT))
        n2_pool = top.enter_context(tc.tile_pool(name="n2", bufs=DT))
        k1_pool = top.enter_context(tc.tile_pool(name="k1", bufs=DT))
        m2_pool = top.enter_context(tc.tile_pool(name="m2", bufs=DT))
        id_pool = top.enter_context(tc.tile_pool(name="idp", bufs=1))
        wq_pool = top.enter_context(tc.tile_pool(name="wq", bufs=DT))
        wk_pool = top.enter_context(tc.tile_pool(name="wk", bufs=DT))
        wv_pool = top.enter_context(tc.tile_pool(name="wv", bufs=DT))
        wo_pool = top.enter_context(tc.tile_pool(name="wo", bufs=DT))
        xt_pool = top.enter_context(tc.tile_pool(name="xt", bufs=DT))
        ost_pool = top.enter_context(tc.tile_pool(name="ost", bufs=8))
        ps_pool = top.enter_context(tc.tile_pool(name="ps", bufs=8, space="PSUM"))

        # HAM warmup: the PE clock gate defaults to 4/8 (1.2 GHz) and opens
        # only after ~3.4us of sustained activity. Real work can't start until
        # the first DMAs land (~12us), so burn the idle window on tiny
        # matmuls over a zeroed tile; N1 then runs at 2.4 GHz from its first
        # instruction.
        wu_pool = top.enter_context(tc.tile_pool(name="wup", bufs=1))
        wu = wu_pool.tile([P, 64], bf16, name="wu", tag="wu")
        nc.vector.memset(wu[:], 0.0)
        wps = ps_pool.tile([P, 64], f32, name="wps", tag="ps")
        for i in range(64):
            nc.tensor.matmul(
                wps[:64, :], wu[:], wu[:], start=(i == 0), stop=(i == 63)
            )

        gsb = [gsb_pool.tile([P, D], bf16, name=f"g{i}", tag="gsb") for i in range(DT)]
        n1 = [n1_pool.tile([P, FH], bf16, name=f"n1_{i}", tag="n1") for i in range(DT)]
        n2 = [n2_pool.tile([P, FH], bf16, name=f"n2_{i}", tag="n2") for i in range(DT)]
        k1 = [k1_pool.tile([P, FH], bf16, name=f"k1_{i}", tag="k1") for i in range(DT)]
        m2 = [m2_pool.tile([P, FH], bf16, name=f"m2_{i}", tag="m2") for i in range(DT)]
        idt = id_pool.tile([P, P], bf16, name="idt", tag="idt")

        # DMA queues: gpsimd carries wvT then xt group 0 (N1 is the tensor
        # engine's opening act); woh rides sync ahead of xn; xn splits across
        # sync/scalar so G is never DMA-starved; wk/wqT follow behind. xt
        # groups 1-3 recycle xn's SBUF space once G is done.
        wvb = [wv_pool.tile([P, D], bf16, name=f"wv{i}", tag="wv") for i in range(DT)]
        wob = [wo_pool.tile([P, FH], bf16, name=f"wo{i}", tag="wo") for i in range(DT)]
        for i in range(DT):
            nc.gpsimd.dma_start(out=wvb[i][:], in_=wvT[ts(i, P), :])
            nc.sync.dma_start(out=wob[i][:], in_=woh[ts(i, P), :])
        nc.gpsimd.dma_start(out=idt[:], in_=ident)  # only needed at ~100us
        xtg_all = [
            [xt_pool.tile([P, D], bf16, name=f"xt0_{i}", tag="xt") for i in range(DT)]
        ]
        for it in range(DT):
            nc.gpsimd.dma_start(out=xtg_all[0][it][:], in_=xt[ts(it, P), ts(0, D)])

        with contextlib.ExitStack() as setup:
            xn_pool = setup.enter_context(tc.tile_pool(name="xn", bufs=ST))
            xnb = [xn_pool.tile([P, D], bf16, name=f"xn{i}", tag="xn") for i in range(ST)]
            for i in range(ST):
                q = nc.sync if i % 2 == 0 else nc.scalar
                q.dma_start(out=xnb[i][:], in_=xn[ts(i, P), :])
            wkb = [wk_pool.tile([P, D], bf16, name=f"wk{i}", tag="wk") for i in range(DT)]
            wqb = [wq_pool.tile([P, D], bf16, name=f"wq{i}", tag="wq") for i in range(DT)]
            for i in range(DT):
                nc.sync.dma_start(out=wkb[i][:], in_=wk[ts(i, P), :])
                nc.scalar.dma_start(out=wqb[i][:], in_=wqT[ts(i, P), :])

            # --- N1 = wv @ wo[:, half] (tensor warms up while xn streams) ---
            # Two rounds of 4 PSUM banks so the first half's evacs overlap the
            # second half's matmuls; otherwise G-A's first matmuls serialize
            # behind all 8 evacuations draining through the vector engine.
            for half in range(2):
                pss = [
                    ps_pool.tile([P, FREE], f32, name="psn1", tag="ps")
                    for _ in range(4)
                ]
                for dt in range(DT):
                    for e4 in range(4):
                        nc.tensor.matmul(
                            pss[e4][:],
                            wvb[dt][:, ts(half * 4 + e4, P)],
                            wob[dt][:],
                            start=(dt == 0),
                            stop=(dt == DT - 1),
                        )
                for e4 in range(4):
                    nc.vector.tensor_copy(n1[half * 4 + e4][:], pss[e4][:])

            # --- G upper triangle: two streaming passes over the sequence ---
            for chunks in G_PASS:
                pss = {
                    (jt, c): ps_pool.tile([P, FREE], f32, name="psg", tag="ps")
                    for (jt, c, es, w) in chunks
                }
                for tt in range(ST):
                    for jt, c, es, w in chunks:
                        nc.tensor.matmul(
                            pss[jt, c][:, :w],
                            xnb[tt][:, ts(jt, P)],
                            xnb[tt][:, es : es + w],
                            start=(tt == 0),
                            stop=(tt == ST - 1),
                        )
                for jt, c, es, w in chunks:
                    nc.vector.tensor_copy(gsb[jt][:, es : es + w], pss[jt, c][:, :w])

            # --- mirror the lower-triangle blocks: G[jt,eb] = G[eb,jt].T ---
            for jt in range(1, DT):
                for eb in range(jt):
                    pst = ps_pool.tile([P, P], bf16, name="pst", tag="ps")
                    nc.tensor.transpose(pst[:], gsb[eb][:, ts(jt, P)], idt[:])
                    nc.vector.tensor_copy(gsb[jt][:, ts(eb, P)], pst[:])

        # xt groups 1-3 into the SBUF space xn vacated.
        xt2_pool = top.enter_context(tc.tile_pool(name="xt2", bufs=3 * DT))
        for g in range(1, 4):
            xtg = [
                xt2_pool.tile([P, D], bf16, name=f"xt{g}_{i}", tag="xt2")
                for i in range(DT)
            ]
            for it in range(DT):
                nc.gpsimd.dma_start(out=xtg[it][:], in_=xt[ts(it, P), ts(g, D)])
            xtg_all.append(xtg)

        # --- N2 = G @ N1 (lhsT=G works because G is symmetric) ---
        pss = [ps_pool.tile([P, FREE], f32, name="psn2", tag="ps") for _ in range(DT)]
        for et in range(DT):
            for db in range(DT):
                nc.tensor.matmul(
                    pss[db][:],
                    gsb[et][:, ts(db, P)],
                    n1[et][:],
                    start=(et == 0),
                    stop=(et == DT - 1),
                )
        for db in range(DT):
            nc.scalar.copy(n2[db][:], pss[db][:])

        # --- K1 = wk.T @ N2 ---
        pss = [ps_pool.tile([P, FREE], f32, name="psk1", tag="ps") for _ in range(DT)]
        for dt in range(DT):
            for cb in range(DT):
                nc.tensor.matmul(
                    pss[cb][:],
                    wkb[dt][:, ts(cb, P)],
                    n2[dt][:],
                    start=(dt == 0),
                    stop=(dt == DT - 1),
                )
        for cb in range(DT):
            nc.vector.tensor_copy(k1[cb][:], pss[cb][:])

        # --- M2 = wq @ K1 ---
        pss = [ps_pool.tile([P, FREE], f32, name="psm2", tag="ps") for _ in range(DT)]
        for ct in range(DT):
            for ib in range(DT):
                nc.tensor.matmul(
                    pss[ib][:],
                    wqb[ct][:, ts(ib, P)],
                    k1[ct][:],
                    start=(ct == 0),
                    stop=(ct == DT - 1),
                )
        for ib in range(DT):
            nc.scalar.copy(m2[ib][:], pss[ib][:])

        # --- out[:, half] = x @ M2, streaming xt column-groups of 1024 ---
        for g in range(4):
            xtg = xtg_all[g]
            pss = [ps_pool.tile([P, FREE], f32, name="pso", tag="ps") for _ in range(8)]
            for it in range(DT):
                for tb in range(8):
                    nc.tensor.matmul(
                        pss[tb][:],
                        xtg[it][:, ts(tb, P)],
                        m2[it][:],
                        start=(it == 0),
                        stop=(it == DT - 1),
                    )
            # Tail latency: alternate evac engines and store queues so the
            # last group's flush isn't serialized on one engine.
            for tb in range(8):
                ot = ost_pool.tile([P, FREE], f32, name="ot", tag="ost")
                if tb % 2 == 0:
                    nc.scalar.copy(ot[:], pss[tb][:])
                    nc.scalar.dma_start(out=out[ts(g * 8 + tb, P), :], in_=ot[:])
                else:
                    nc.vector.tensor_copy(ot[:], pss[tb][:])
                    nc.sync.dma_start(out=out[ts(g * 8 + tb, P), :], in_=ot[:])


def _build():
    _install_axon_ntff_shim()
    import concourse.mybir as mybir
    import concourse.tile as tile
    from concourse import bacc

    f32 = mybir.dt.float32
    bf16 = mybir.dt.bfloat16
    nc = bacc.Bacc("TRN2", target_bir_lowering=False, debug=False, num_devices=NCORES)
    xn = nc.dram_tensor("xn", [T, D], bf16, kind="ExternalInput").ap()
    xt = nc.dram_tensor("xt", [D, T], bf16, kind="ExternalInput").ap()
    wqT = nc.dram_tensor("wqT", [D, D], bf16, kind="ExternalInput").ap()
    wk = nc.dram_tensor("wk", [D, D], bf16, kind="ExternalInput").ap()
    wvT = nc.dram_tensor("wvT", [D, D], bf16, kind="ExternalInput").ap()
    woh = nc.dram_tensor("woh", [D, FH], bf16, kind="ExternalInput").ap()
    ident = nc.dram_tensor("ident", [P, P], bf16, kind="ExternalInput").ap()
    out = nc.dram_tensor("out", [T, FH], f32, kind="ExternalOutput").ap()

    with tile.TileContext(nc) as tc:
        _trace_kernel(tc, xn, xt, wqT, wk, wvT, woh, ident, out)
    nc.compile()
    return nc


def kernel(x, w_q, w_k, w_v, w_o):
    global LAST_RESULTS
    import ml_dtypes
    from concourse import bass_utils

    bf16 = ml_dtypes.bfloat16

    if "nc" not in _STATE:
        _STATE["nc"] = _build()
    nc = _STATE["nc"]

    x = np.ascontiguousarray(x, dtype=np.float32)
    wqT = np.ascontiguousarray(np.asarray(w_q, dtype=np.float32).T).astype(bf16)
    wkb = np.ascontiguousarray(np.asarray(w_k, dtype=np.float32)).astype(bf16)
    wvT = np.ascontiguousarray(np.asarray(w_v, dtype=np.float32).T).astype(bf16)
    wo = np.asarray(w_o, dtype=np.float32)
    wo_halves = [
        np.ascontiguousarray(wo[:, :FH]).astype(bf16),
        np.ascontiguousarray(wo[:, FH:]).astype(bf16),
    ]
    ident = np.eye(P, dtype=np.float32).astype(bf16)
    xn_b = [x[b].astype(bf16) for b in range(B)]
    xt_b = [np.ascontiguousarray(x[b].T).astype(bf16) for b in range(B)]

    in_maps = []
    for core in range(NCORES):
        b, fh = core // 2, core % 2
        in_maps.append(
            {
                "xn": xn_b[b],
                "xt": xt_b[b],
                "wqT": wqT,
                "wk": wkb,
                "wvT": wvT,
                "woh": wo_halves[fh],
                "ident": ident,
            }
        )

    LAST_RESULTS = bass_utils.run_bass_kernel_spmd(
        nc, in_maps, core_ids=list(range(NCORES))
    )
    out = np.empty((B, T, D), dtype=np.float32)
    for core in range(NCORES):
        b, fh = core // 2, core % 2
        out[b, :, fh * FH : (fh + 1) * FH] = LAST_RESULTS.results[core]["out"]
    return out
